# revision 4
# baseline (speedup 1.0000x reference)
"""Trainium2 Bass kernel for BlockDecomposedSSMAttention.

Math: y[b,s,:] = x[b,s,:] @ B.T @ A @ C.T   (no cross-block recurrence)
 ==>  y = x @ W  with  W = B.T @ A @ C.T

Distribution over the 8 NeuronCores (grid = 2 row-groups x 4 col-quarters):
  core c = (rg, cq):  computes y[rg*8192:(rg+1)*8192, cq*256:(cq+1)*256]
  - x rows are split 2 ways (8192 rows/core, read by 4 cores each).
  - Each core only needs W[:, cq*256:(cq+1)*256], so the W-build stages
    shrink 4x vs full-W-per-core:  T = A @ C.T[:, quarter]   (1024x256)
                                   W_q = B.T @ T             (1024x256)

Timing model (per core, measured: preamble ~7us, first DMA byte ~8.4us,
one ~360 GB/s input stream shared by all queues, 216ns/512-row matmul):
  - params (ct 0.5 + at 2 + bt 2 MB) stream 8.4->20.9us; build is
    DMA-paced inside that window (stage1 ends ~17, stage2 ~22.6).
    The duplicated build (32768 cyc) is the right size for this window;
    splitting it across cores would only idle the PE here.
  - last bt piece is kept small so only 8 matmuls trail the last byte.
  - x (16 MB) streams JIT behind params under the 55.3us main loop.
  - main loop: W stationary, x moving at N=512, 4 psum banks per
    m-group; y.T out via gpsimd SWDGE (off the input queue), final
    groups drained via the (by then idle) sync HWDGE for a short tail.
  - All matmul operands bf16 (1 cyc/row), PSUM fp32; y written bf16,
    host transposes + upcasts. Host does layout marshalling only.
"""

import os
import sys

import numpy as np

if "/opt/trn_rl_repo" not in sys.path:
    sys.path.insert(0, "/opt/trn_rl_repo")

import ml_dtypes

BF16 = ml_dtypes.bfloat16

BATCH, SEQ, D = 4, 4096, 1024
NCORES = 8
RG, CQ = 2, 4                 # row-groups x col-quarters
ROWS = BATCH * SEQ            # 16384
MSH = ROWS // RG              # 8192 rows per core
OD = D // CQ                  # 256 output cols per core
P = 128
KT = D // P                   # 8 contraction tiles
MC = 512                      # moving chunk of m in the main loop
NMC = MSH // MC               # 16 m-chunks
NOT = OD // P                 # 2 o'-tiles

_CACHE: dict = {}


def _build_nc():
    import concourse.mybir as mybir
    import concourse.tile as tile
    from concourse import bacc

    f32 = mybir.dt.float32
    bf16 = mybir.dt.bfloat16

    nc = bacc.Bacc(
        "TRN2", target_bir_lowering=False, debug=False, num_devices=NCORES,
        num_swdge_queues=1,
    )

    # Per-core inputs (bf16, contraction dim on partitions):
    #   at [kp, ko, j]  = A[j, ko*128+kp]            (A.T, replicated)
    #   bt [jp, jo, i]  = B[jo*128+jp, i]            (B,   replicated)
    #   ct [kp, ko, o]  = C[cq*256+o, ko*128+kp]     (C.T col-quarter)
    #   xt [ip, io, m]  = x2[rg*8192+m, io*128+ip]   (x row-shard, transposed)
    at_in = nc.dram_tensor("at_in", [P, KT, D], bf16, kind="ExternalInput")
    b_in = nc.dram_tensor("b_in", [P, KT, D], bf16, kind="ExternalInput")
    ct_in = nc.dram_tensor("ct_in", [P, KT, OD], bf16, kind="ExternalInput")
    xt = nc.dram_tensor("xt", [P, KT, MSH], bf16, kind="ExternalInput")
    # y.T shard [o', m]; host transposes + upcasts.
    y_out = nc.dram_tensor("y_out", [OD, MSH], bf16, kind="ExternalOutput")

    with tile.TileContext(nc) as tc:
        with (
            tc.tile_pool(name="big", bufs=1) as big,
            tc.tile_pool(name="ycopy", bufs=8) as ycopy,
            tc.tile_pool(name="psp", bufs=8, space="PSUM") as psp,
        ):
            at_sb = big.tile([P, KT, D], bf16)
            ct_sb = big.tile([P, KT, OD], bf16)
            bt_sb = big.tile([P, KT, D], bf16)
            t_sb = big.tile([P, KT, OD], bf16)
            w_sb = big.tile([P, KT, OD], bf16)
            xt_sb = big.tile([P, KT, MSH], bf16)

            # ---- input DMAs, all on the single Sync HWDGE queue in exact
            # consumption order (the queue is FIFO and the ~360 GB/s input
            # bandwidth does not aggregate across queues, so one well-ordered
            # stream is optimal). y output uses GpSimd/SWDGE so its
            # completions never flow-control late input chunks.
            def dget(t_sb_, t_in_, sl):
                nc.sync.dma_start(t_sb_[(slice(None),) + sl],
                                  t_in_.ap()[(slice(None),) + sl])

            # stage-1 feed: ct+at, small first pieces for an early PE start
            dget(ct_sb, ct_in, (slice(0, 1),))
            dget(at_sb, at_in, (slice(0, 1),))
            dget(ct_sb, ct_in, (slice(1, 4),))
            dget(at_sb, at_in, (slice(1, 4),))
            dget(ct_sb, ct_in, (slice(4, 8),))
            dget(at_sb, at_in, (slice(4, 8),))
            # stage-2 feed: bt; last piece small so only jt=7's 8 matmuls
            # trail the final byte
            dget(bt_sb, b_in, (slice(0, 4),))
            dget(bt_sb, b_in, (slice(4, 7),))
            dget(bt_sb, b_in, (slice(7, 8),))
            # x stream: two small chunks (first m-group), then 2 MiB chunks
            dget(xt_sb, xt, (slice(None), slice(0, 512)))
            dget(xt_sb, xt, (slice(None), slice(512, 1024)))
            for c in range(1, 8):
                dget(xt_sb, xt, (slice(None), slice(1024 * c, 1024 * (c + 1))))

            # ---- stage 1: T = A @ Ct_q  [1024 x 256], kt-outer over a
            # single 8-bank pass (j-tiles 0..7), paced by the at stream.
            ps1 = [psp.tile([P, MC], f32, name="psp") for j in range(KT)]
            for kt in range(KT):
                for jt in range(KT):
                    nc.tensor.matmul(
                        ps1[jt][:, 0:OD],
                        at_sb[:, kt, jt * P : (jt + 1) * P],
                        ct_sb[:, kt, :],
                        start=(kt == 0),
                        stop=(kt == KT - 1),
                    )
            for jt in range(KT):
                if jt % 2 == 0:
                    nc.vector.tensor_copy(t_sb[:, jt, :], ps1[jt][:, 0:OD])
                else:
                    nc.scalar.copy(t_sb[:, jt, :], ps1[jt][:, 0:OD])

            # ---- stage 2: W_q = B.T @ T  [1024 x 256], jt-outer over a
            # single 8-bank pass (it-tiles 0..7), paced by the bt stream.
            ps2 = [psp.tile([P, MC], f32, name="psp") for i in range(KT)]
            for jt in range(KT):
                for it in range(KT):
                    nc.tensor.matmul(
                        ps2[it][:, 0:OD],
                        bt_sb[:, jt, it * P : (it + 1) * P],
                        t_sb[:, jt, :],
                        start=(jt == 0),
                        stop=(jt == KT - 1),
                    )
            for it in range(KT):
                if it % 2 == 0:
                    nc.vector.tensor_copy(w_sb[:, it, :], ps2[it][:, 0:OD])
                else:
                    nc.scalar.copy(w_sb[:, it, :], ps2[it][:, 0:OD])

            # ---- main: y_q.T = W_q.T @ x.T  [256 x 8192] ----
            # W stationary (reused across m), x moving at N=512.
            # groups: pairs of 512-chunks (= one 2 MiB x DMA chunk) -> 4 psum
            # banks per group; final chunk split 256+256 for a short drain
            # tail, with its y DMAs on the (by then idle) sync HWDGE.
            groups = [[(MC * 2 * g, MC), (MC * (2 * g + 1), MC)]
                      for g in range(NMC // 2 - 1)]
            groups += [[(MSH - 1024, MC), (MSH - 512, 256)], [(MSH - 256, 256)]]
            for gi, chunks in enumerate(groups):
                last = gi == len(groups) - 1
                pms = [
                    psp.tile([P, MC], f32, name="psp")
                    for i in range(len(chunks) * NOT)
                ]
                for ot in range(NOT):
                    for it in range(KT):
                        for ci, (m0, ml) in enumerate(chunks):
                            nc.tensor.matmul(
                                pms[len(chunks) * ot + ci][:, 0:ml],
                                w_sb[:, it, ot * P : (ot + 1) * P],
                                xt_sb[:, it, m0 : m0 + ml],
                                start=(it == 0),
                                stop=(it == KT - 1),
                            )
                for ot in range(NOT):
                    for ci, (m0, ml) in enumerate(chunks):
                        yt = ycopy.tile([P, MC], bf16, name="yt")
                        # alternate copy engines so drains never queue behind
                        # each other; last group: vector+scalar in parallel,
                        # y out on the (by then idle) sync HWDGE.
                        if last and ot == 1:
                            nc.scalar.copy(
                                yt[:, 0:ml], pms[len(chunks) * ot + ci][:, 0:ml]
                            )
                        else:
                            nc.vector.tensor_copy(
                                yt[:, 0:ml], pms[len(chunks) * ot + ci][:, 0:ml]
                            )
                        eng = nc.sync if last else nc.gpsimd
                        eng.dma_start(
                            y_out.ap()[ot * P : (ot + 1) * P, m0 : m0 + ml],
                            yt[:, 0:ml],
                        )

    nc.compile()
    return nc


def _get_nc():
    if "nc" not in _CACHE:
        _CACHE["nc"] = _build_nc()
    return _CACHE["nc"]


def _make_in_maps(x, A, B, C):
    x2 = np.ascontiguousarray(x, dtype=np.float32).reshape(ROWS, D)
    at = np.ascontiguousarray(
        np.asarray(A, np.float32).reshape(D, KT, P).transpose(2, 1, 0)
    ).astype(BF16)
    bt = np.ascontiguousarray(
        np.asarray(B, np.float32).reshape(KT, P, D).transpose(1, 0, 2)
    ).astype(BF16)
    xts = []
    for rg in range(RG):
        shard = x2[rg * MSH : (rg + 1) * MSH]  # [MSH, D]
        xts.append(
            np.ascontiguousarray(
                shard.reshape(MSH, KT, P).transpose(2, 1, 0)
            ).astype(BF16)
        )
    in_maps = []
    for c in range(NCORES):
        rg, cq = divmod(c, CQ)
        csl = np.asarray(C, np.float32)[cq * OD : (cq + 1) * OD, :]  # [OD, D]
        ct = np.ascontiguousarray(
            csl.T.reshape(KT, P, OD).transpose(1, 0, 2)
        ).astype(BF16)
        in_maps.append({"at_in": at, "b_in": bt, "ct_in": ct, "xt": xts[rg]})
    return in_maps


def _install_ntff_hook():
    """The agent image's ``antenv`` lacks ``axon_hooks``; recreate it and
    register the ctypes-based NTFF profile hook (same as trn_boot's
    ``_ntff_profile_via_ctypes``) so ``trace=True`` yields exec_time_ns."""
    import contextlib
    import ctypes
    import types

    if "antenv.axon_hooks" in sys.modules:
        return True
    so_path = "/opt/axon/libaxon_pjrt.so"
    if not os.path.exists(so_path):
        return False
    lib = ctypes.CDLL(so_path)
    if not hasattr(lib, "axon_start_nrt_profile"):
        return False
    lib.axon_start_nrt_profile.argtypes = [
        ctypes.POINTER(ctypes.c_int64),
        ctypes.c_size_t,
    ]
    lib.axon_start_nrt_profile.restype = ctypes.c_int64
    lib.axon_stop_nrt_profile.argtypes = [ctypes.c_char_p]
    lib.axon_stop_nrt_profile.restype = ctypes.c_int64

    @contextlib.contextmanager
    def _hook(output_dir, device_ids):
        import jax

        jax.devices()
        if device_ids:
            ids = (ctypes.c_int64 * len(device_ids))(*device_ids)
            rc = lib.axon_start_nrt_profile(ids, len(device_ids))
        else:
            rc = lib.axon_start_nrt_profile(None, 0)
        if rc != 0:
            raise RuntimeError(f"axon_start_nrt_profile rc={rc}")
        try:
            yield
        finally:
            n = lib.axon_stop_nrt_profile(str(output_dir).encode())
            print(f"ntff profile: {n} file(s) written to {output_dir}")

    mod = types.ModuleType("antenv.axon_hooks")
    _state = {"hook": _hook}
    mod.set_axon_ntff_profile_hook = lambda h: _state.__setitem__("hook", h)
    mod.get_axon_ntff_profile_hook = lambda: _state["hook"]
    sys.modules["antenv.axon_hooks"] = mod
    import antenv

    antenv.axon_hooks = mod
    return True


def run(x, A, B, C, trace=False):
    """Run on hardware; returns (y_full, exec_time_ns_or_None)."""
    from concourse import bass_utils
    from concourse.bass_interp import get_hw_module

    if trace and not _install_ntff_hook():
        trace = False
    if trace:
        # upload_artifacts pushes the NEFF dir to a remote bucket; in this
        # sandbox that can fail AFTER a successful run, losing the results.
        # Degrade to the local path. (Only touches the tracing dev path.)
        if not getattr(bass_utils.upload_artifacts, "_safe", False):
            _orig_upload = bass_utils.upload_artifacts

            def _safe_upload(tmpdir):
                try:
                    return _orig_upload(tmpdir)
                except Exception as e:
                    print(f"upload_artifacts skipped ({type(e).__name__}): {e}")
                    return str(tmpdir)

            _safe_upload._safe = True
            bass_utils.upload_artifacts = _safe_upload

    nc = _get_nc()
    in_maps = _make_in_maps(x, A, B, C)

    old_m = nc.m
    nc.m = get_hw_module(nc.m)
    try:
        res = bass_utils.run_bass_kernel_spmd(
            nc, in_maps, core_ids=list(range(NCORES)), trace=trace
        )
    finally:
        nc.m = old_m

    y2 = np.empty((ROWS, D), dtype=np.float32)
    for c in range(NCORES):
        rg, cq = divmod(c, CQ)
        y2[rg * MSH : (rg + 1) * MSH, cq * OD : (cq + 1) * OD] = (
            res.results[c]["y_out"].T.astype(np.float32)
        )
    return y2.reshape(BATCH, SEQ, D), res.exec_time_ns


def kernel(x, A, B, C):
    y, _ = run(x, A, B, C, trace=False)
    return y


# revision 5
# speedup vs baseline: 1.0267x; 1.0267x over previous
"""Trainium2 Bass kernel for BlockDecomposedSSMAttention.

Math: y[b,s,:] = x[b,s,:] @ B.T @ A @ C.T   (no cross-block recurrence)
 ==>  y = x @ W  with  W = B.T @ A @ C.T

Distribution over the 8 NeuronCores (grid = 2 row-groups x 4 col-quarters):
  core c = (rg, cq):  computes y[rg*8192:(rg+1)*8192, cq*256:(cq+1)*256]
  - x rows are split 2 ways (8192 rows/core, read by 4 cores each).
  - Each core only needs W[:, cq*256:(cq+1)*256], so the W-build stages
    shrink 4x vs full-W-per-core:  T = A @ C.T[:, quarter]   (1024x256)
                                   W_q = B.T @ T             (1024x256)

Timing model (per core, measured: preamble ~7us, first DMA byte ~8.4us,
one ~360 GB/s input stream shared by all queues, 216ns/512-row matmul):
  - params (ct 0.5 + at 2 + bt 2 MB) stream 8.4->20.9us; build is
    DMA-paced inside that window (stage1 ends ~17, stage2 ~22.6).
    The duplicated build (32768 cyc) is the right size for this window;
    splitting it across cores would only idle the PE here.
  - last bt piece is kept small so only 8 matmuls trail the last byte.
  - x (16 MB) streams JIT behind params under the 55.3us main loop.
  - main loop: W stationary, x moving at N=512, 4 psum banks per
    m-group; y.T out via gpsimd SWDGE (off the input queue), final
    groups drained via the (by then idle) sync HWDGE for a short tail.
  - All matmul operands bf16 (1 cyc/row), PSUM fp32; y written bf16,
    host transposes + upcasts. Host does layout marshalling only.
"""

import os
import sys

import numpy as np

if "/opt/trn_rl_repo" not in sys.path:
    sys.path.insert(0, "/opt/trn_rl_repo")

import ml_dtypes

BF16 = ml_dtypes.bfloat16

BATCH, SEQ, D = 4, 4096, 1024
NCORES = 8
RG, CQ = 2, 4                 # row-groups x col-quarters
ROWS = BATCH * SEQ            # 16384
MSH = ROWS // RG              # 8192 rows per core
OD = D // CQ                  # 256 output cols per core
P = 128
KT = D // P                   # 8 contraction tiles
MC = 512                      # moving chunk of m in the main loop
NMC = MSH // MC               # 16 m-chunks
NOT = OD // P                 # 2 o'-tiles

_CACHE: dict = {}


def _build_nc():
    import concourse.mybir as mybir
    import concourse.tile as tile
    from concourse import bacc

    f32 = mybir.dt.float32
    bf16 = mybir.dt.bfloat16

    nc = bacc.Bacc(
        "TRN2", target_bir_lowering=False, debug=False, num_devices=NCORES,
        num_swdge_queues=1,
    )

    # Per-core inputs (bf16, contraction dim on partitions):
    #   at [kp, ko, j]  = A[j, ko*128+kp]            (A.T, replicated)
    #   bt [jp, jo, i]  = B[jo*128+jp, i]            (B,   replicated)
    #   ct [kp, ko, o]  = C[cq*256+o, ko*128+kp]     (C.T col-quarter)
    #   xt [ip, io, m]  = x2[rg*8192+m, io*128+ip]   (x row-shard, transposed)
    at_in = nc.dram_tensor("at_in", [P, KT, D], bf16, kind="ExternalInput")
    b_in = nc.dram_tensor("b_in", [P, KT, D], bf16, kind="ExternalInput")
    ct_in = nc.dram_tensor("ct_in", [P, KT, OD], bf16, kind="ExternalInput")
    xt = nc.dram_tensor("xt", [P, KT, MSH], bf16, kind="ExternalInput")
    # y.T shard [o', m]; host transposes + upcasts.
    y_out = nc.dram_tensor("y_out", [OD, MSH], bf16, kind="ExternalOutput")

    with tile.TileContext(nc) as tc:
        with (
            tc.tile_pool(name="big", bufs=1) as big,
            tc.tile_pool(name="ycopy", bufs=8) as ycopy,
            tc.tile_pool(name="psp", bufs=8, space="PSUM") as psp,
        ):
            at_sb = big.tile([P, KT, D], bf16)
            ct_sb = big.tile([P, KT, OD], bf16)
            bt_sb = big.tile([P, KT, D], bf16)
            t_sb = big.tile([P, KT, OD], bf16)
            w_sb = big.tile([P, KT, OD], bf16)
            xt_sb = big.tile([P, KT, MSH], bf16)

            # ---- input DMAs, all on the single Sync HWDGE queue in exact
            # consumption order (the queue is FIFO and the ~360 GB/s input
            # bandwidth does not aggregate across queues, so one well-ordered
            # stream is optimal). y output uses GpSimd/SWDGE so its
            # completions never flow-control late input chunks.
            def dget(t_sb_, t_in_, sl):
                nc.sync.dma_start(t_sb_[(slice(None),) + sl],
                                  t_in_.ap()[(slice(None),) + sl])

            # stage-1 feed: ct+at, small first pieces for an early PE start
            dget(ct_sb, ct_in, (slice(0, 2),))
            dget(at_sb, at_in, (slice(0, 2),))
            dget(ct_sb, ct_in, (slice(2, 5),))
            dget(at_sb, at_in, (slice(2, 5),))
            dget(ct_sb, ct_in, (slice(5, 8),))
            dget(at_sb, at_in, (slice(5, 8),))
            # stage-2 feed: bt; last piece small so only jt=7's 8 matmuls
            # trail the final byte
            dget(bt_sb, b_in, (slice(0, 4),))
            dget(bt_sb, b_in, (slice(4, 7),))
            dget(bt_sb, b_in, (slice(7, 8),))
            # x stream: two small chunks (first m-group), then 2 MiB chunks
            dget(xt_sb, xt, (slice(None), slice(0, 512)))
            dget(xt_sb, xt, (slice(None), slice(512, 1024)))
            for c in range(1, 8):
                dget(xt_sb, xt, (slice(None), slice(1024 * c, 1024 * (c + 1))))

            # ---- stage 1: T = A @ Ct_q  [1024 x 256], kt-outer over a
            # single 8-bank pass (j-tiles 0..7), paced by the at stream.
            ps1 = [psp.tile([P, MC], f32, name="psp") for j in range(KT)]
            for kt in range(KT):
                for jt in range(KT):
                    nc.tensor.matmul(
                        ps1[jt][:, 0:OD],
                        at_sb[:, kt, jt * P : (jt + 1) * P],
                        ct_sb[:, kt, :],
                        start=(kt == 0),
                        stop=(kt == KT - 1),
                    )
            for jt in range(KT):
                if jt % 2 == 0:
                    nc.vector.tensor_copy(t_sb[:, jt, :], ps1[jt][:, 0:OD])
                else:
                    nc.scalar.copy(t_sb[:, jt, :], ps1[jt][:, 0:OD])

            # ---- stage 2: W_q = B.T @ T  [1024 x 256], jt-outer over a
            # single 8-bank pass (it-tiles 0..7), paced by the bt stream.
            ps2 = [psp.tile([P, MC], f32, name="psp") for i in range(KT)]
            for jt in range(KT):
                for it in range(KT):
                    nc.tensor.matmul(
                        ps2[it][:, 0:OD],
                        bt_sb[:, jt, it * P : (it + 1) * P],
                        t_sb[:, jt, :],
                        start=(jt == 0),
                        stop=(jt == KT - 1),
                    )
            for it in range(KT):
                if it % 2 == 0:
                    nc.vector.tensor_copy(w_sb[:, it, :], ps2[it][:, 0:OD])
                else:
                    nc.scalar.copy(w_sb[:, it, :], ps2[it][:, 0:OD])

            # ---- main: y_q.T = W_q.T @ x.T  [256 x 8192] ----
            # W stationary (reused across m), x moving at N=512.
            # groups: pairs of 512-chunks (= one 2 MiB x DMA chunk) -> 4 psum
            # banks per group; final chunk split 256+256 for a short drain
            # tail, with its y DMAs on the (by then idle) sync HWDGE.
            groups = [[(MC * 2 * g, MC), (MC * (2 * g + 1), MC)]
                      for g in range(NMC // 2 - 1)]
            groups += [[(MSH - 1024, MC), (MSH - 512, 256)], [(MSH - 256, 256)]]
            for gi, chunks in enumerate(groups):
                last = gi == len(groups) - 1
                pms = [
                    psp.tile([P, MC], f32, name="psp")
                    for i in range(len(chunks) * NOT)
                ]
                for ot in range(NOT):
                    for it in range(KT):
                        for ci, (m0, ml) in enumerate(chunks):
                            nc.tensor.matmul(
                                pms[len(chunks) * ot + ci][:, 0:ml],
                                w_sb[:, it, ot * P : (ot + 1) * P],
                                xt_sb[:, it, m0 : m0 + ml],
                                start=(it == 0),
                                stop=(it == KT - 1),
                            )
                for ot in range(NOT):
                    for ci, (m0, ml) in enumerate(chunks):
                        yt = ycopy.tile([P, MC], bf16, name="yt")
                        # alternate copy engines so drains never queue behind
                        # each other; last group: vector+scalar in parallel,
                        # y out on the (by then idle) sync HWDGE.
                        if last and ot == 1:
                            nc.scalar.copy(
                                yt[:, 0:ml], pms[len(chunks) * ot + ci][:, 0:ml]
                            )
                        else:
                            nc.vector.tensor_copy(
                                yt[:, 0:ml], pms[len(chunks) * ot + ci][:, 0:ml]
                            )
                        eng = nc.sync if last else nc.gpsimd
                        eng.dma_start(
                            y_out.ap()[ot * P : (ot + 1) * P, m0 : m0 + ml],
                            yt[:, 0:ml],
                        )

    nc.compile()
    return nc


def _get_nc():
    if "nc" not in _CACHE:
        _CACHE["nc"] = _build_nc()
    return _CACHE["nc"]


def _make_in_maps(x, A, B, C):
    x2 = np.ascontiguousarray(x, dtype=np.float32).reshape(ROWS, D)
    at = np.ascontiguousarray(
        np.asarray(A, np.float32).reshape(D, KT, P).transpose(2, 1, 0)
    ).astype(BF16)
    bt = np.ascontiguousarray(
        np.asarray(B, np.float32).reshape(KT, P, D).transpose(1, 0, 2)
    ).astype(BF16)
    xts = []
    for rg in range(RG):
        shard = x2[rg * MSH : (rg + 1) * MSH]  # [MSH, D]
        xts.append(
            np.ascontiguousarray(
                shard.reshape(MSH, KT, P).transpose(2, 1, 0)
            ).astype(BF16)
        )
    in_maps = []
    for c in range(NCORES):
        rg, cq = divmod(c, CQ)
        csl = np.asarray(C, np.float32)[cq * OD : (cq + 1) * OD, :]  # [OD, D]
        ct = np.ascontiguousarray(
            csl.T.reshape(KT, P, OD).transpose(1, 0, 2)
        ).astype(BF16)
        in_maps.append({"at_in": at, "b_in": bt, "ct_in": ct, "xt": xts[rg]})
    return in_maps


def _install_ntff_hook():
    """The agent image's ``antenv`` lacks ``axon_hooks``; recreate it and
    register the ctypes-based NTFF profile hook (same as trn_boot's
    ``_ntff_profile_via_ctypes``) so ``trace=True`` yields exec_time_ns."""
    import contextlib
    import ctypes
    import types

    if "antenv.axon_hooks" in sys.modules:
        return True
    so_path = "/opt/axon/libaxon_pjrt.so"
    if not os.path.exists(so_path):
        return False
    lib = ctypes.CDLL(so_path)
    if not hasattr(lib, "axon_start_nrt_profile"):
        return False
    lib.axon_start_nrt_profile.argtypes = [
        ctypes.POINTER(ctypes.c_int64),
        ctypes.c_size_t,
    ]
    lib.axon_start_nrt_profile.restype = ctypes.c_int64
    lib.axon_stop_nrt_profile.argtypes = [ctypes.c_char_p]
    lib.axon_stop_nrt_profile.restype = ctypes.c_int64

    @contextlib.contextmanager
    def _hook(output_dir, device_ids):
        import jax

        jax.devices()
        if device_ids:
            ids = (ctypes.c_int64 * len(device_ids))(*device_ids)
            rc = lib.axon_start_nrt_profile(ids, len(device_ids))
        else:
            rc = lib.axon_start_nrt_profile(None, 0)
        if rc != 0:
            raise RuntimeError(f"axon_start_nrt_profile rc={rc}")
        try:
            yield
        finally:
            n = lib.axon_stop_nrt_profile(str(output_dir).encode())
            print(f"ntff profile: {n} file(s) written to {output_dir}")

    mod = types.ModuleType("antenv.axon_hooks")
    _state = {"hook": _hook}
    mod.set_axon_ntff_profile_hook = lambda h: _state.__setitem__("hook", h)
    mod.get_axon_ntff_profile_hook = lambda: _state["hook"]
    sys.modules["antenv.axon_hooks"] = mod
    import antenv

    antenv.axon_hooks = mod
    return True


def run(x, A, B, C, trace=False):
    """Run on hardware; returns (y_full, exec_time_ns_or_None)."""
    from concourse import bass_utils
    from concourse.bass_interp import get_hw_module

    if trace and not _install_ntff_hook():
        trace = False
    if trace:
        # upload_artifacts pushes the NEFF dir to a remote bucket; in this
        # sandbox that can fail AFTER a successful run, losing the results.
        # Degrade to the local path. (Only touches the tracing dev path.)
        if not getattr(bass_utils.upload_artifacts, "_safe", False):
            _orig_upload = bass_utils.upload_artifacts

            def _safe_upload(tmpdir):
                try:
                    return _orig_upload(tmpdir)
                except Exception as e:
                    print(f"upload_artifacts skipped ({type(e).__name__}): {e}")
                    return str(tmpdir)

            _safe_upload._safe = True
            bass_utils.upload_artifacts = _safe_upload

    nc = _get_nc()
    in_maps = _make_in_maps(x, A, B, C)

    old_m = nc.m
    nc.m = get_hw_module(nc.m)
    try:
        res = bass_utils.run_bass_kernel_spmd(
            nc, in_maps, core_ids=list(range(NCORES)), trace=trace
        )
    finally:
        nc.m = old_m

    y2 = np.empty((ROWS, D), dtype=np.float32)
    for c in range(NCORES):
        rg, cq = divmod(c, CQ)
        y2[rg * MSH : (rg + 1) * MSH, cq * OD : (cq + 1) * OD] = (
            res.results[c]["y_out"].T.astype(np.float32)
        )
    return y2.reshape(BATCH, SEQ, D), res.exec_time_ns


def kernel(x, A, B, C):
    y, _ = run(x, A, B, C, trace=False)
    return y


# revision 6
# speedup vs baseline: 1.0460x; 1.0188x over previous
"""Trainium2 Bass kernel for BlockDecomposedSSMAttention.

Math: y[b,s,:] = x[b,s,:] @ B.T @ A @ C.T   (no cross-block recurrence)
 ==>  y = x @ W  with  W = B.T @ A @ C.T

Distribution over the 8 NeuronCores (grid = 2 row-groups x 4 col-quarters):
  core c = (rg, cq):  computes y[rg*8192:(rg+1)*8192, cq*256:(cq+1)*256]
  - x rows are split 2 ways (8192 rows/core, read by 4 cores each).
  - Each core only needs W[:, cq*256:(cq+1)*256], so the W-build stages
    shrink 4x vs full-W-per-core:  T = A @ Ct_q   (1024x256)
                                   W_q = B.T @ T  (1024x256)

Timing model (measured: preamble ~7us, first DMA byte ~8.4us, per-core
input stream capped ~350 GB/s with no cross-queue aggregation,
216ns/512-row matmul; collectives are 5-10x degraded here, so the
duplicated per-core build beats any cross-core W sharing):
  - params are HOST-PACKED into one tensor in exact consumption order
    (per kt: ct 256 cols | at 1024 cols, then bt), so one small first
    piece (0.31 MB, 2.5-KB lines) starts the PE at ~10us and the build
    runs PE-bound to ~24us with zero issue-order stalls.
  - x is chunk-major ([P, chunk, kt, m] -> 8-KB DMA lines) and streams
    JIT behind params under the 55.4us main loop.
  - main loop: W stationary, x moving at N=512, 4 psum banks per
    m-group; y.T out via gpsimd SWDGE (off the input queue). y_out is
    [P, ot, m] so the final 256-row group drains with ONE sync-HWDGE
    DMA right after its two (vector+scalar, parallel) psum copies.
  - All matmul operands bf16 (1 cyc/row), PSUM fp32; y written bf16,
    host transposes + upcasts. Host does layout marshalling only.
"""

import os
import sys

import numpy as np

if "/opt/trn_rl_repo" not in sys.path:
    sys.path.insert(0, "/opt/trn_rl_repo")

import ml_dtypes

BF16 = ml_dtypes.bfloat16

BATCH, SEQ, D = 4, 4096, 1024
NCORES = 8
RG, CQ = 2, 4                 # row-groups x col-quarters
ROWS = BATCH * SEQ            # 16384
MSH = ROWS // RG              # 8192 rows per core
OD = D // CQ                  # 256 output cols per core
P = 128
KT = D // P                   # 8 contraction tiles
MC = 512                      # moving chunk of m in the main loop
NMC = MSH // MC               # 16 m-chunks
NOT = OD // P                 # 2 o'-tiles

KTW = OD + D                  # 1280 packed param cols per kt (ct | at)
BTO = KT * KTW                # 10240: offset of the bt section
PW = BTO + KT * D             # 18432 packed param cols total

_CACHE: dict = {}


def _build_nc():
    import concourse.mybir as mybir
    import concourse.tile as tile
    from concourse import bacc

    f32 = mybir.dt.float32
    bf16 = mybir.dt.bfloat16

    nc = bacc.Bacc(
        "TRN2", target_bir_lowering=False, debug=False, num_devices=NCORES,
        num_swdge_queues=1,
    )

    # Per-core inputs (bf16, contraction dim on partitions):
    #   pk [kp, kt*1280 + o]        = C[cq*256+o, kt*128+kp]   (ct slice)
    #   pk [kp, kt*1280 + 256 + j]  = A[j, kt*128+kp]          (A.T)
    #   pk [jp, 10240 + jt*1024 + i] = B[jt*128+jp, i]         (B)
    #   xt [ip, c, io, m] = x2[rg*8192 + c*512 + m, io*128+ip] (x shard)
    pk_in = nc.dram_tensor("pk_in", [P, PW], bf16, kind="ExternalInput")
    xt = nc.dram_tensor("xt", [P, NMC, KT, MC], bf16, kind="ExternalInput")
    # y.T shard as [p, o', m]; host transposes + upcasts.
    y_out = nc.dram_tensor("y_out", [P, NOT, MSH], bf16, kind="ExternalOutput")

    def ct_ap(sb, kt):
        return sb[:, kt * KTW : kt * KTW + OD]

    def at_ap(sb, kt, jt):
        o = kt * KTW + OD + jt * P
        return sb[:, o : o + P]

    def bt_ap(sb, jt, it):
        o = BTO + jt * D + it * P
        return sb[:, o : o + P]

    with tile.TileContext(nc) as tc:
        with (
            tc.tile_pool(name="big", bufs=1) as big,
            tc.tile_pool(name="ycopy", bufs=8) as ycopy,
            tc.tile_pool(name="psp", bufs=8, space="PSUM") as psp,
        ):
            pk_sb = big.tile([P, PW], bf16)
            t_sb = big.tile([P, KT, OD], bf16)
            w_sb = big.tile([P, KT, OD], bf16)
            xt_sb = big.tile([P, NMC, KT, MC], bf16)

            # ---- input DMAs, all on the single Sync HWDGE queue in exact
            # consumption order (FIFO; per-core bandwidth doesn't aggregate
            # across queues, so one well-ordered stream is optimal). y goes
            # out via GpSimd/SWDGE so its completions never flow-control
            # late input chunks.
            def pget(lo, hi):
                nc.sync.dma_start(pk_sb[:, lo:hi], pk_in.ap()[:, lo:hi])

            pget(0, KTW)                  # kt0: first matmul ~2us earlier
            pget(KTW, 3 * KTW)            # kt 1-2
            pget(3 * KTW, 5 * KTW)        # kt 3-4
            pget(5 * KTW, BTO)            # kt 5-7
            pget(BTO, BTO + 4 * D)        # bt jt 0-3
            pget(BTO + 4 * D, BTO + 7 * D)  # bt jt 4-6
            pget(BTO + 7 * D, PW)         # bt jt7 small: short post-byte tail
            # x stream: two 1-chunk pieces (first m-group), then 2 MiB pairs
            nc.sync.dma_start(xt_sb[:, 0:1], xt.ap()[:, 0:1])
            nc.sync.dma_start(xt_sb[:, 1:2], xt.ap()[:, 1:2])
            for c in range(1, 8):
                nc.sync.dma_start(
                    xt_sb[:, 2 * c : 2 * c + 2], xt.ap()[:, 2 * c : 2 * c + 2]
                )

            # ---- stage 1: T = A @ Ct_q  [1024 x 256], kt-outer over a
            # single 8-bank pass (j-tiles 0..7), paced by the packed stream.
            ps1 = [psp.tile([P, MC], f32, name="psp") for j in range(KT)]
            for kt in range(KT):
                for jt in range(KT):
                    nc.tensor.matmul(
                        ps1[jt][:, 0:OD],
                        at_ap(pk_sb, kt, jt),
                        ct_ap(pk_sb, kt),
                        start=(kt == 0),
                        stop=(kt == KT - 1),
                    )
            for jt in range(KT):
                if jt % 2 == 0:
                    nc.vector.tensor_copy(t_sb[:, jt, :], ps1[jt][:, 0:OD])
                else:
                    nc.scalar.copy(t_sb[:, jt, :], ps1[jt][:, 0:OD])

            # ---- stage 2: W_q = B.T @ T  [1024 x 256], jt-outer over a
            # single 8-bank pass (it-tiles 0..7), paced by the bt stream.
            ps2 = [psp.tile([P, MC], f32, name="psp") for i in range(KT)]
            for jt in range(KT):
                for it in range(KT):
                    nc.tensor.matmul(
                        ps2[it][:, 0:OD],
                        bt_ap(pk_sb, jt, it),
                        t_sb[:, jt, :],
                        start=(jt == 0),
                        stop=(jt == KT - 1),
                    )
            for it in range(KT):
                if it % 2 == 0:
                    nc.vector.tensor_copy(w_sb[:, it, :], ps2[it][:, 0:OD])
                else:
                    nc.scalar.copy(w_sb[:, it, :], ps2[it][:, 0:OD])

            # ---- main: y_q.T = W_q.T @ x.T  [256 x 8192] ----
            # W stationary (reused across m), x moving at N=512.
            # groups: pairs of 512-chunks (= one 2 MiB x DMA piece) -> 4 psum
            # banks per group; final group is a single 256-row chunk drained
            # by ONE dma on the (by then idle) sync HWDGE.
            groups = [[(MC * 2 * g, MC), (MC * (2 * g + 1), MC)]
                      for g in range(NMC // 2 - 1)]
            groups += [[(MSH - 1024, MC), (MSH - 512, 256)], [(MSH - 256, 256)]]
            for gi, chunks in enumerate(groups):
                last = gi == len(groups) - 1
                pms = [
                    psp.tile([P, MC], f32, name="psp")
                    for i in range(len(chunks) * NOT)
                ]
                for ot in range(NOT):
                    for it in range(KT):
                        for ci, (m0, ml) in enumerate(chunks):
                            cc, off = divmod(m0, MC)
                            nc.tensor.matmul(
                                pms[len(chunks) * ot + ci][:, 0:ml],
                                w_sb[:, it, ot * P : (ot + 1) * P],
                                xt_sb[:, cc, it, off : off + ml],
                                start=(it == 0),
                                stop=(it == KT - 1),
                            )
                if last:
                    (m0, ml) = chunks[0]
                    yl = ycopy.tile([P, NOT, 256], bf16, name="ylast")
                    nc.vector.tensor_copy(yl[:, 0, :], pms[0][:, 0:ml])
                    nc.scalar.copy(yl[:, 1, :], pms[1][:, 0:ml])
                    nc.sync.dma_start(y_out.ap()[:, :, m0 : m0 + ml], yl[:])
                else:
                    for ot in range(NOT):
                        for ci, (m0, ml) in enumerate(chunks):
                            yt = ycopy.tile([P, MC], bf16, name="yt")
                            nc.vector.tensor_copy(
                                yt[:, 0:ml], pms[len(chunks) * ot + ci][:, 0:ml]
                            )
                            nc.gpsimd.dma_start(
                                y_out.ap()[:, ot, m0 : m0 + ml], yt[:, 0:ml]
                            )

    nc.compile()
    return nc


def _get_nc():
    if "nc" not in _CACHE:
        _CACHE["nc"] = _build_nc()
    return _CACHE["nc"]


def _make_in_maps(x, A, B, C):
    x2 = np.ascontiguousarray(x, dtype=np.float32).reshape(ROWS, D)
    at = np.asarray(A, np.float32).reshape(D, KT, P).transpose(2, 1, 0)  # [P,KT,D]
    bt = np.asarray(B, np.float32).reshape(KT, P, D).transpose(1, 0, 2)  # [P,KT,D]
    xts = []
    for rg in range(RG):
        shard = x2[rg * MSH : (rg + 1) * MSH]  # [MSH, D]
        xts.append(
            np.ascontiguousarray(
                shard.reshape(NMC, MC, KT, P).transpose(3, 0, 2, 1)
            ).astype(BF16)
        )
    in_maps = []
    for c in range(NCORES):
        rg, cq = divmod(c, CQ)
        csl = np.asarray(C, np.float32)[cq * OD : (cq + 1) * OD, :]  # [OD, D]
        ct = csl.T.reshape(KT, P, OD).transpose(1, 0, 2)  # [P,KT,OD]
        pk = np.empty((P, PW), dtype=np.float32)
        for kt in range(KT):
            pk[:, kt * KTW : kt * KTW + OD] = ct[:, kt, :]
            pk[:, kt * KTW + OD : (kt + 1) * KTW] = at[:, kt, :]
        for jt in range(KT):
            pk[:, BTO + jt * D : BTO + (jt + 1) * D] = bt[:, jt, :]
        in_maps.append({"pk_in": pk.astype(BF16), "xt": xts[rg]})
    return in_maps


def _install_ntff_hook():
    """The agent image's ``antenv`` lacks ``axon_hooks``; recreate it and
    register the ctypes-based NTFF profile hook (same as trn_boot's
    ``_ntff_profile_via_ctypes``) so ``trace=True`` yields exec_time_ns."""
    import contextlib
    import ctypes
    import types

    if "antenv.axon_hooks" in sys.modules:
        return True
    so_path = "/opt/axon/libaxon_pjrt.so"
    if not os.path.exists(so_path):
        return False
    lib = ctypes.CDLL(so_path)
    if not hasattr(lib, "axon_start_nrt_profile"):
        return False
    lib.axon_start_nrt_profile.argtypes = [
        ctypes.POINTER(ctypes.c_int64),
        ctypes.c_size_t,
    ]
    lib.axon_start_nrt_profile.restype = ctypes.c_int64
    lib.axon_stop_nrt_profile.argtypes = [ctypes.c_char_p]
    lib.axon_stop_nrt_profile.restype = ctypes.c_int64

    @contextlib.contextmanager
    def _hook(output_dir, device_ids):
        import jax

        jax.devices()
        if device_ids:
            ids = (ctypes.c_int64 * len(device_ids))(*device_ids)
            rc = lib.axon_start_nrt_profile(ids, len(device_ids))
        else:
            rc = lib.axon_start_nrt_profile(None, 0)
        if rc != 0:
            raise RuntimeError(f"axon_start_nrt_profile rc={rc}")
        try:
            yield
        finally:
            n = lib.axon_stop_nrt_profile(str(output_dir).encode())
            print(f"ntff profile: {n} file(s) written to {output_dir}")

    mod = types.ModuleType("antenv.axon_hooks")
    _state = {"hook": _hook}
    mod.set_axon_ntff_profile_hook = lambda h: _state.__setitem__("hook", h)
    mod.get_axon_ntff_profile_hook = lambda: _state["hook"]
    sys.modules["antenv.axon_hooks"] = mod
    import antenv

    antenv.axon_hooks = mod
    return True


def run(x, A, B, C, trace=False):
    """Run on hardware; returns (y_full, exec_time_ns_or_None)."""
    from concourse import bass_utils
    from concourse.bass_interp import get_hw_module

    if trace and not _install_ntff_hook():
        trace = False
    if trace:
        # upload_artifacts pushes the NEFF dir to a remote bucket; in this
        # sandbox that can fail AFTER a successful run, losing the results.
        # Degrade to the local path. (Only touches the tracing dev path.)
        if not getattr(bass_utils.upload_artifacts, "_safe", False):
            _orig_upload = bass_utils.upload_artifacts

            def _safe_upload(tmpdir):
                try:
                    return _orig_upload(tmpdir)
                except Exception as e:
                    print(f"upload_artifacts skipped ({type(e).__name__}): {e}")
                    return str(tmpdir)

            _safe_upload._safe = True
            bass_utils.upload_artifacts = _safe_upload

    nc = _get_nc()
    in_maps = _make_in_maps(x, A, B, C)

    old_m = nc.m
    nc.m = get_hw_module(nc.m)
    try:
        res = bass_utils.run_bass_kernel_spmd(
            nc, in_maps, core_ids=list(range(NCORES)), trace=trace
        )
    finally:
        nc.m = old_m

    y2 = np.empty((ROWS, D), dtype=np.float32)
    for c in range(NCORES):
        rg, cq = divmod(c, CQ)
        arr = res.results[c]["y_out"]  # [P, NOT, MSH]
        yT = arr.transpose(1, 0, 2).reshape(OD, MSH)
        y2[rg * MSH : (rg + 1) * MSH, cq * OD : (cq + 1) * OD] = (
            yT.T.astype(np.float32)
        )
    return y2.reshape(BATCH, SEQ, D), res.exec_time_ns


def kernel(x, A, B, C):
    y, _ = run(x, A, B, C, trace=False)
    return y
